# revision 6
# baseline (speedup 1.0000x reference)
"""CapsNet dense routing kernel for 8 Trainium2 NeuronCores.

Problem: capsule routing with 3 iterations (last skips the logit update).
  u_hat[b,n,u,v] = sum_k W[n,u,k,v] * x[b,n,k]        (B=128, N=2048, U=32, K=8, V=16)
  repeat:  c = softmax(b_logit, axis=u)
           s[b,u,v] = sum_n c[n,u] u_hat[b,n,u,v]
           v = squash(s)
           b_logit[n,u] += sum_{b,v} u_hat[b,n,u,v] v[b,u,v]

Strategy: shard n (in_caps) across the 8 cores (256 each).  u_hat is never
materialized:
  - s is one fused matmul  s[b,(uv)] = sum_(nk) xT[(nk),b] * (c*W)[(nk),(uv)]
    with only the n-partial sum needing an 8-core all-reduce per iteration.
  - the logit update uses P[(nk),(uv)] = sum_b x[b,(nk)] v[b,(uv)] (a matmul),
    then b_delta[n,u] = sum_{k,v} P*W  via an elementwise multiply, a
    v-reduction, and one block-diagonal "sum over k" matmul.

The all-reduce avoids the collectives firmware entirely (its ncfw control
plane costs 15-35us per op at this message size): each core sends its
[128,512] bf16 partial to each peer with remote_dma_broadcast
(relative-dest slot d -> Delta-tpb d, a XOR bijection, so every receiver
gets each sender exactly once across its 7 landing slots), then reduces
the 8 partials locally on the vector engine in fp32.  Cross-core
synchronization is one semaphore wait per all-reduce, attached to the
first consuming instruction AFTER Tile scheduling (the scheduler's sim
cannot model remote increments).  After the full all-reduce of the last
iteration every core holds the complete output; the host reads core 0's.

Matmul operands travel in bf16 (fp32 matmul costs two PE passes); all
accumulation is fp32.  sqrt inside squash uses a bitcast fast-rsqrt with
Newton steps on the vector engine so ScalarE never leaves the exp table set.
"""

import sys

sys.path.insert(0, "/opt/trn_rl_repo")

import ml_dtypes
import numpy as np

B, N, U, K, V, NC = 128, 2048, 32, 8, 16, 8
NSH = N // NC            # 256 in_caps per core
T = NSH * K // 128       # 16 contraction tiles of 128 (n,k) rows
UV = U * V               # 512
BU = T * U               # 512 free size of the k-replicated logit buffer
GRP = 2                  # P-tiles per PSUM round
RSQRT_MAGIC = 0x5F3759DF
AR_INC = 14              # remote_sem increments per all-reduce (7 senders x 2)
LS_INC = 112             # local_sem increments per all-reduce (7 x 16)

_cache = {}


def _build_program():
    import concourse.mybir as mybir
    import concourse.tile as tile
    from concourse import bacc

    fp32 = mybir.dt.float32
    bf16 = mybir.dt.bfloat16

    nc = bacc.Bacc(
        "TRN2", target_bir_lowering=False, debug=False, num_devices=NC
    )
    xT_d = nc.dram_tensor("xT", [128, T * B], bf16, kind="ExternalInput").ap()
    xn_d = nc.dram_tensor("xn", [B, T * 128], bf16, kind="ExternalInput").ap()
    wp_d = nc.dram_tensor("wp", [128, T * UV], bf16, kind="ExternalInput").ap()
    s2_d = nc.dram_tensor("s2", [128, 128], bf16, kind="ExternalInput").ap()
    vo_d = nc.dram_tensor("vout", [B, UV], fp32, kind="ExternalOutput").ap()
    dbg_d = nc.dram_tensor("dbg", [1, 8], fp32, kind="ExternalOutput").ap()

    rsem = nc.alloc_semaphore("rsem")
    lsem = nc.alloc_semaphore("lsem")
    post_waits = []

    with tile.TileContext(nc) as tc:
        _body(tc, nc, mybir, fp32, bf16, xT_d, xn_d, wp_d, s2_d, vo_d, dbg_d,
              rsem, lsem, post_waits)
    # Cross-core semaphore waits go in AFTER Tile scheduling: the scheduling
    # sim has no model of remote increments and would deadlock on them.
    # compile()'s generate_event_semaphores splits multi-wait instructions.
    for inst, sem, val in post_waits:
        inst.wait_op(sem, val, "sem-ge", check=False)
    nc.compile()
    return nc


def _squash(nc, mybir, sm, fp32, bass, s_in, v_out, s_scale, pp, tag,
            newton=2):
    """v_out = squash(s_in * s_scale); s_in [pp, UV] viewed [pp, U, V].

    factor = sqrt(n2)/(1+n2) with n2 = scale^2 * sum_v s^2; all on DVE:
    rsqrt seed by integer bitcast, Newton steps, no ScalarE tables.
    """
    Alu = mybir.AluOpType
    sq = sm.tile([pp, UV], fp32, tag=f"sq{tag}")
    n2 = sm.tile([pp, U], fp32, tag=f"n2{tag}")
    y = sm.tile([pp, U], fp32, tag=f"y{tag}")
    t = sm.tile([pp, U], fp32, tag=f"t{tag}")
    d_ = sm.tile([pp, U], fp32, tag=f"d_{tag}")
    rd_ = sm.tile([pp, U], fp32, tag=f"rd_{tag}")
    fac = sm.tile([pp, U], fp32, tag=f"fac{tag}")
    nc.vector.tensor_mul(sq[:, : UV // 2], s_in[:, : UV // 2],
                         s_in[:, : UV // 2])
    nc.vector.tensor_mul(sq[:, UV // 2 :], s_in[:, UV // 2 :],
                         s_in[:, UV // 2 :])
    nc.vector.reduce_sum(
        out=n2, in_=sq.rearrange("p (u v) -> p u v", v=V),
        axis=mybir.AxisListType.X,
    )
    if s_scale != 1.0:
        nc.vector.tensor_scalar_mul(out=n2, in0=n2,
                                    scalar1=float(s_scale * s_scale))
    # y0 = bitcast(0x5F3759DF - (bitcast(n2) >> 1))
    nc.vector.tensor_scalar(
        out=y.bitcast(mybir.dt.int32), in0=n2.bitcast(mybir.dt.int32),
        scalar1=1, scalar2=None, op0=Alu.logical_shift_right,
    )
    nc.vector.tensor_scalar(
        out=y.bitcast(mybir.dt.int32), in0=y.bitcast(mybir.dt.int32),
        scalar1=-1, scalar2=RSQRT_MAGIC, op0=Alu.mult, op1=Alu.add,
    )
    for _ in range(newton):  # Newton: y *= 1.5 - 0.5*n2*y*y
        nc.vector.tensor_mul(t, y, y)
        nc.vector.tensor_mul(t, t, n2)
        nc.vector.tensor_scalar(
            out=t, in0=t, scalar1=-0.5, scalar2=1.5, op0=Alu.mult, op1=Alu.add
        )
        nc.vector.tensor_mul(y, y, t)
    # d = 1+n2 ; fac = (n2*scale)*y / d      ((n2*y) = sqrt(n2))
    nc.vector.tensor_scalar_add(out=d_, in0=n2, scalar1=1.0)
    nc.vector.reciprocal(out=rd_, in_=d_)
    nc.vector.scalar_tensor_tensor(
        out=t, in0=n2, scalar=float(s_scale), in1=y,
        op0=Alu.mult, op1=Alu.mult,
    )
    nc.vector.tensor_mul(fac, t, rd_)
    fac_b = bass.AP(
        tensor=fac.tensor, offset=fac.offset,
        ap=[fac.ap[0], [1, U], [0, V]],
    )
    nc.vector.tensor_mul(
        v_out.rearrange("p (u v) -> p u v", v=V),
        s_in.rearrange("p (u v) -> p u v", v=V),
        fac_b,
    )


def _body(tc, nc, mybir, fp32, bf16, xT_d, xn_d, wp_d, s2_d, vo_d, dbg_d,
          rsem, lsem, post_waits):
    from contextlib import ExitStack

    import concourse.bass as bass

    AF = mybir.ActivationFunctionType
    NG = T // GRP            # fused pipeline groups per iteration
    GU = GRP * U             # 64  b-logit columns per group
    GUV = GRP * UV           # 1024 wc columns per group

    ctx = ExitStack()
    tc._caps_ctx = ctx
    sing = ctx.enter_context(tc.tile_pool(name="sing", bufs=1))
    wcp = ctx.enter_context(tc.tile_pool(name="wcp", bufs=2))
    qp = ctx.enter_context(tc.tile_pool(name="qp", bufs=1))
    sm = ctx.enter_context(tc.tile_pool(name="sm", bufs=2))
    ps_s = ctx.enter_context(tc.tile_pool(name="ps_s", bufs=2, space="PSUM"))
    ps_p = ctx.enter_context(tc.tile_pool(name="ps_p", bufs=2, space="PSUM"))
    ps_b = ctx.enter_context(tc.tile_pool(name="ps_b", bufs=2, space="PSUM"))

    # preload the exp table set during the DMA ramp
    actwarm = sing.tile([1, 1], fp32)
    nc.vector.memset(actwarm, 1.0)
    nc.scalar.activation(out=actwarm, in_=actwarm, func=AF.Exp)

    # ---- resident inputs ---------------------------------------------------
    xT_sb = sing.tile([128, T * B], bf16)
    xn_sb = sing.tile([B, T * 128], bf16)
    wp_sb = sing.tile([128, T * UV], bf16)
    s2_sb = sing.tile([128, 128], bf16)
    b_exp = sing.tile([128, BU], fp32)

    nc.sync.dma_start(out=s2_sb, in_=s2_d)
    # few, large chunks: each sync.dma_start costs ~0.6us of issue time
    nc.sync.dma_start(out=xT_sb[:, : 8 * B], in_=xT_d[:, : 8 * B])
    for c4 in range(4):
        w = 4 * UV
        nc.sync.dma_start(
            out=wp_sb[:, c4 * w : (c4 + 1) * w],
            in_=wp_d[:, c4 * w : (c4 + 1) * w],
        )
    nc.sync.dma_start(out=xT_sb[:, 8 * B :], in_=xT_d[:, 8 * B :])
    nc.sync.dma_start(out=xn_sb, in_=xn_d)
    nc.vector.memset(b_exp, 0.0)

    def reduce_s(s_psum, it):
        """Remote-DMA all-reduce of the [B,UV] n-partial sum.

        PSUM -> bf16 SBUF -> 7 relative-dest broadcasts (slot d, XOR
        bijection) -> fp32 accumulate of the 8 partials on DVE.  Returns
        the fp32 SBUF tile holding the full sum."""
        s_part = sing.tile([B, UV], bf16, name=f"s_part{it}")
        land = sing.tile([B, 7 * UV], bf16, name=f"land{it}")
        H = UV // 2
        nc.scalar.copy(out=s_part[:, :H], in_=s_psum[:, :H])
        nc.scalar.copy(out=s_part[:, H:], in_=s_psum[:, H:])
        for d in range(1, 8):
            rd = [None] * 8
            rd[d] = (0, d)
            nc.gpsimd.remote_dma_broadcast(
                land[:, (d - 1) * UV : d * UV], s_part[:, :],
                rsem, lsem, rdests=rd,
            )
        # No lsem wait: send-completion increments trickle in extremely
        # slowly (dummy-lane descriptors), and the 21-broadcast total fits
        # the SWDGE ring; receivers' rsem waits gate all data hazards.
        nc.gpsimd.trigger_dma(count=None)
        s_sb = sm.tile([B, UV], fp32, tag="s_acc", name=f"s_acc{it}")
        a = nc.vector.tensor_add(s_sb, s_part, land[:, 0:UV])
        post_waits.append((a, rsem, AR_INC * (it + 1)))
        for d in range(2, 8):
            nc.vector.tensor_add(s_sb, s_sb, land[:, (d - 1) * UV : d * UV])
        return s_sb, s_part

    # ---- iteration 0: c uniform, s0 = sum_t xT_t.T @ wp_t ------------------
    s_psum = ps_s.tile([B, UV], fp32, tag="s")
    for t in range(T):
        nc.tensor.matmul(
            out=s_psum,
            lhsT=xT_sb[:, t * B : (t + 1) * B],
            rhs=wp_sb[:, t * UV : (t + 1) * UV],
            start=(t == 0),
            stop=(t == T - 1),
        )
    s_sb, s_part = reduce_s(s_psum, 0)

    # ---- post-AR fused pipeline for iterations 1 and 2 ---------------------
    warm_tiles = []
    for i in range(2):
        # PE clock warm-up: the HAM gate needs ~3.4us of dense activity to
        # lift the PE from 1.2 to 2.4 GHz.  These accumulating matmuls are
        # gated on the local bf16 partial, so they fill the otherwise
        # PE-idle all-reduce + squash window and the burst then runs warm.
        warm_ps = ps_b.tile([128, BU], fp32, tag="bd", name=f"warm{i}")
        for w in range(8):
            nc.tensor.matmul(out=warm_ps, lhsT=wp_sb[:, :B], rhs=s_part,
                             start=(w == 0), stop=(w == 7))
        warm_tiles.append(warm_ps)
        s_scale = 1.0 / U if i == 0 else 1.0
        v_bf = sm.tile([B, UV], bf16, tag="v_bf", name=f"v_bf{i}")
        _squash(nc, mybir, sm, fp32, bass, s_sb, v_bf, s_scale, B, f"i{i}",
                newton=1)

        q_sb = qp.tile([128, T * UV], bf16, tag="q")
        qr_bf = sm.tile([128, BU], bf16, tag="qr_bf")
        e_sb = sm.tile([128, BU], fp32, tag="e")
        rden = sm.tile([128, T], fp32, tag="rden")
        cx_sb = wcp.tile([128, T * UV], bf16, tag="cx")
        wc_sb = wcp.tile([128, T * UV], bf16, tag="wc")
        bd_psum = ps_b.tile([128, BU], fp32, tag="bd")
        s_psum = ps_s.tile([B, UV], fp32, tag="s")

        for g in range(NG):
            lo_t = g * GRP
            cbu = slice(g * GU, (g + 1) * GU)      # b/c columns
            cwc = slice(g * GUV, (g + 1) * GUV)    # wc columns
            # b_delta for these GRP tiles
            p_psum = ps_p.tile([128, GUV], fp32, tag="p")
            p_bf = sm.tile([128, GUV], bf16, tag="p_bf")
            for j in range(GRP):
                t = lo_t + j
                nc.tensor.matmul(
                    out=p_psum[:, j * UV : (j + 1) * UV],
                    lhsT=xn_sb[:, t * 128 : (t + 1) * 128],
                    rhs=v_bf,
                    start=True,
                    stop=True,
                )
            nc.scalar.copy(out=p_bf, in_=p_psum)
            nc.vector.tensor_mul(q_sb[:, cwc], p_bf, wp_sb[:, cwc])
            with nc.allow_low_precision(reason="qr feeds fp32 PSUM matmul"):
                nc.vector.reduce_sum(
                    out=qr_bf[:, cbu],
                    in_=q_sb[:, cwc].rearrange("p (a v) -> p a v", v=V),
                    axis=mybir.AxisListType.X,
                )
            nc.tensor.matmul(
                out=bd_psum[:, cbu], lhsT=s2_sb, rhs=qr_bf[:, cbu],
                start=True, stop=True,
            )
            nc.vector.tensor_add(b_exp[:, cbu], b_exp[:, cbu], bd_psum[:, cbu])
            # softmax for these tiles, folded into the weights
            nc.scalar.activation(out=e_sb[:, cbu], in_=b_exp[:, cbu],
                                 func=AF.Exp)
            nc.vector.reduce_sum(
                out=rden[:, g * GRP : (g + 1) * GRP],
                in_=e_sb[:, cbu].rearrange("p (t u) -> p t u", u=U),
                axis=mybir.AxisListType.X,
            )
            nc.vector.reciprocal(
                out=rden[:, g * GRP : (g + 1) * GRP],
                in_=rden[:, g * GRP : (g + 1) * GRP],
            )
            # c expanded over v on ScalarE (c = e * 1/den, per-partition
            # scale per tile); unit-stride bf16 result lets the Wc multiply
            # hit the DVE 2x mode
            for j in range(GRP):
                t = lo_t + j
                ev = e_sb[:, t * U : (t + 1) * U]
                e_b = bass.AP(tensor=ev.tensor, offset=ev.offset,
                              ap=[ev.ap[0], [1, U], [0, V]])
                cx_out = cx_sb[:, t * UV : (t + 1) * UV].rearrange(
                    "p (u v) -> p u v", v=V
                )
                nc.scalar.activation(out=cx_out, in_=e_b, func=AF.Copy,
                                     scale=rden[:, t : t + 1])
            nc.vector.tensor_mul(wc_sb[:, cwc], wp_sb[:, cwc], cx_sb[:, cwc])
            for j in range(GRP):
                t = lo_t + j
                nc.tensor.matmul(
                    out=s_psum,
                    lhsT=xT_sb[:, t * B : (t + 1) * B],
                    rhs=wc_sb[:, t * UV : (t + 1) * UV],
                    start=(t == 0),
                    stop=(t == T - 1),
                )
        s_sb, s_part = reduce_s(s_psum, i + 1)

    # ---- final squash: every core has the full sum; host reads core 0 ------
    v_sb = sm.tile([B, UV], fp32, tag="v_sb2")
    _squash(nc, mybir, sm, fp32, bass, s_sb, v_sb, 1.0, B, "2", newton=2)
    nc.sync.dma_start(out=vo_d, in_=v_sb)
    wseed = sm.tile([1, 8], fp32, tag="wseed")
    for k, wt in enumerate(warm_tiles):
        nc.scalar.copy(out=wseed[:, k * 4 : (k + 1) * 4], in_=wt[0:1, 0:4])
    nc.sync.dma_start(out=dbg_d, in_=wseed)

    ctx.close()


def _host_prep(x, W):
    """Slice + relayout the full inputs into the 8 per-core input maps."""
    bf = ml_dtypes.bfloat16
    S2 = np.zeros((128, 128), np.float32)
    for p in range(0, 128, 8):
        S2[p : p + 8, p : p + 8] = 1.0
    S2 = S2.astype(bf)
    maps = []
    for c in range(NC):
        sl = slice(c * NSH, (c + 1) * NSH)
        Wc = np.ascontiguousarray(W[sl])                    # [256,32,8,16]
        Wp = Wc.transpose(0, 2, 1, 3).reshape(NSH * K, UV)  # [(n k),(u v)]
        wp = Wp.reshape(T, 128, UV).transpose(1, 0, 2).reshape(128, T * UV)
        xc = np.ascontiguousarray(x[:, sl, :])              # [128,256,8]
        xn = xc.reshape(B, NSH * K)
        xT = (
            xc.transpose(1, 2, 0)
            .reshape(T, 128, B)
            .transpose(1, 0, 2)
            .reshape(128, T * B)
        )
        maps.append(
            {
                "xT": np.ascontiguousarray(xT).astype(bf),
                "xn": np.ascontiguousarray(xn).astype(bf),
                "wp": np.ascontiguousarray(wp).astype(bf),
                "s2": S2,
            }
        )
    return maps


def kernel(x: np.ndarray, W: np.ndarray) -> np.ndarray:
    from concourse.bass_utils import run_bass_kernel_spmd

    if "nc" not in _cache:
        _cache["nc"] = _build_program()
    nc = _cache["nc"]
    in_maps = _host_prep(np.asarray(x, np.float32), np.asarray(W, np.float32))
    res = run_bass_kernel_spmd(nc, in_maps, core_ids=list(range(NC)))
    return res.results[0]["vout"].reshape(B, U, V).astype(np.float32)


# revision 7
# speedup vs baseline: 31.3255x; 31.3255x over previous
"""CapsNet dense routing kernel for 8 Trainium2 NeuronCores.

Problem: capsule routing with 3 iterations (last skips the logit update).
  u_hat[b,n,u,v] = sum_k W[n,u,k,v] * x[b,n,k]        (B=128, N=2048, U=32, K=8, V=16)
  repeat:  c = softmax(b_logit, axis=u)
           s[b,u,v] = sum_n c[n,u] u_hat[b,n,u,v]
           v = squash(s)
           b_logit[n,u] += sum_{b,v} u_hat[b,n,u,v] v[b,u,v]

Strategy: shard n (in_caps) across the 8 cores (256 each).  u_hat is never
materialized:
  - s is one fused matmul  s[b,(uv)] = sum_(nk) xT[(nk),b] * (c*W)[(nk),(uv)]
    with only the n-partial sum needing an 8-core all-reduce per iteration.
  - the logit update uses P[(nk),(uv)] = sum_b x[b,(nk)] v[b,(uv)] (a matmul),
    then b_delta[n,u] = sum_{k,v} P*W  via an elementwise multiply, a
    v-reduction, and one block-diagonal "sum over k" matmul.

The all-reduce avoids the collectives firmware entirely (its ncfw control
plane costs 15-35us per op at this message size): each core sends its
[128,512] bf16 partial to each peer with remote_dma_broadcast
(relative-dest slot d -> Delta-tpb d, a XOR bijection, so every receiver
gets each sender exactly once across its 7 landing slots), then reduces
the 8 partials locally on the vector engine in fp32.  Cross-core
synchronization is one semaphore wait per all-reduce, attached to the
first consuming instruction AFTER Tile scheduling (the scheduler's sim
cannot model remote increments).  After the full all-reduce of the last
iteration every core holds the complete output; the host reads core 0's.

Matmul operands travel in bf16 (fp32 matmul costs two PE passes); all
accumulation is fp32.  sqrt inside squash uses a bitcast fast-rsqrt with
Newton steps on the vector engine so ScalarE never leaves the exp table set.
"""

import sys

sys.path.insert(0, "/opt/trn_rl_repo")

import ml_dtypes
import numpy as np

B, N, U, K, V, NC = 128, 2048, 32, 8, 16, 8
NSH = N // NC            # 256 in_caps per core
T = NSH * K // 128       # 16 contraction tiles of 128 (n,k) rows
UV = U * V               # 512
BU = T * U               # 512 free size of the k-replicated logit buffer
GRP = 2                  # P-tiles per PSUM round
RSQRT_MAGIC = 0x5F3759DF
AR_INC = 14              # remote_sem increments per all-reduce (7 senders x 2)
LS_INC = 112             # local_sem increments per all-reduce (7 x 16)

_cache = {}


def _build_program():
    import concourse.mybir as mybir
    import concourse.tile as tile
    from concourse import bacc

    fp32 = mybir.dt.float32
    bf16 = mybir.dt.bfloat16

    nc = bacc.Bacc(
        "TRN2", target_bir_lowering=False, debug=False, num_devices=NC
    )
    xT_d = nc.dram_tensor("xT", [128, T * B], bf16, kind="ExternalInput").ap()
    xn_d = nc.dram_tensor("xn", [B, T * 128], bf16, kind="ExternalInput").ap()
    wp_d = nc.dram_tensor("wp", [128, T * UV], bf16, kind="ExternalInput").ap()
    s2_d = nc.dram_tensor("s2", [128, 128], bf16, kind="ExternalInput").ap()
    vo_d = nc.dram_tensor("vout", [B, UV], fp32, kind="ExternalOutput").ap()
    dbg_d = nc.dram_tensor("dbg", [1, 8], fp32, kind="ExternalOutput").ap()

    rsem = nc.alloc_semaphore("rsem")
    lsem = nc.alloc_semaphore("lsem")
    post_waits = []

    with tile.TileContext(nc) as tc:
        _body(tc, nc, mybir, fp32, bf16, xT_d, xn_d, wp_d, s2_d, vo_d, dbg_d,
              rsem, lsem, post_waits)
    # Cross-core semaphore waits go in AFTER Tile scheduling: the scheduling
    # sim has no model of remote increments and would deadlock on them.
    # compile()'s generate_event_semaphores splits multi-wait instructions.
    for inst, sem, val in post_waits:
        inst.wait_op(sem, val, "sem-ge", check=False)
    nc.compile()
    return nc


def _squash(nc, mybir, sm, fp32, bass, s_in, v_out, s_scale, pp, tag,
            newton=2):
    """v_out = squash(s_in * s_scale); s_in [pp, UV] viewed [pp, U, V].

    factor = sqrt(n2)/(1+n2) with n2 = scale^2 * sum_v s^2; all on DVE:
    rsqrt seed by integer bitcast, Newton steps, no ScalarE tables.
    """
    Alu = mybir.AluOpType
    sq = sm.tile([pp, UV], fp32, tag=f"sq{tag}")
    n2 = sm.tile([pp, U], fp32, tag=f"n2{tag}")
    y = sm.tile([pp, U], fp32, tag=f"y{tag}")
    t = sm.tile([pp, U], fp32, tag=f"t{tag}")
    d_ = sm.tile([pp, U], fp32, tag=f"d_{tag}")
    rd_ = sm.tile([pp, U], fp32, tag=f"rd_{tag}")
    fac = sm.tile([pp, U], fp32, tag=f"fac{tag}")
    nc.vector.tensor_mul(sq[:, : UV // 2], s_in[:, : UV // 2],
                         s_in[:, : UV // 2])
    nc.vector.tensor_mul(sq[:, UV // 2 :], s_in[:, UV // 2 :],
                         s_in[:, UV // 2 :])
    nc.vector.reduce_sum(
        out=n2, in_=sq.rearrange("p (u v) -> p u v", v=V),
        axis=mybir.AxisListType.X,
    )
    if s_scale != 1.0:
        nc.vector.tensor_scalar_mul(out=n2, in0=n2,
                                    scalar1=float(s_scale * s_scale))
    # y0 = bitcast(0x5F3759DF - (bitcast(n2) >> 1))
    nc.vector.tensor_scalar(
        out=y.bitcast(mybir.dt.int32), in0=n2.bitcast(mybir.dt.int32),
        scalar1=1, scalar2=None, op0=Alu.logical_shift_right,
    )
    nc.vector.tensor_scalar(
        out=y.bitcast(mybir.dt.int32), in0=y.bitcast(mybir.dt.int32),
        scalar1=-1, scalar2=RSQRT_MAGIC, op0=Alu.mult, op1=Alu.add,
    )
    for _ in range(newton):  # Newton: y *= 1.5 - 0.5*n2*y*y
        nc.vector.tensor_mul(t, y, y)
        nc.vector.tensor_mul(t, t, n2)
        nc.vector.tensor_scalar(
            out=t, in0=t, scalar1=-0.5, scalar2=1.5, op0=Alu.mult, op1=Alu.add
        )
        nc.vector.tensor_mul(y, y, t)
    # d = 1+n2 ; fac = (n2*scale)*y / d      ((n2*y) = sqrt(n2))
    nc.vector.tensor_scalar_add(out=d_, in0=n2, scalar1=1.0)
    nc.vector.reciprocal(out=rd_, in_=d_)
    nc.vector.scalar_tensor_tensor(
        out=t, in0=n2, scalar=float(s_scale), in1=y,
        op0=Alu.mult, op1=Alu.mult,
    )
    nc.vector.tensor_mul(fac, t, rd_)
    fac_b = bass.AP(
        tensor=fac.tensor, offset=fac.offset,
        ap=[fac.ap[0], [1, U], [0, V]],
    )
    nc.vector.tensor_mul(
        v_out.rearrange("p (u v) -> p u v", v=V),
        s_in.rearrange("p (u v) -> p u v", v=V),
        fac_b,
    )


def _body(tc, nc, mybir, fp32, bf16, xT_d, xn_d, wp_d, s2_d, vo_d, dbg_d,
          rsem, lsem, post_waits):
    from contextlib import ExitStack

    import concourse.bass as bass

    AF = mybir.ActivationFunctionType
    NG = T // GRP            # fused pipeline groups per iteration
    GU = GRP * U             # 64  b-logit columns per group
    GUV = GRP * UV           # 1024 wc columns per group

    ctx = ExitStack()
    tc._caps_ctx = ctx
    sing = ctx.enter_context(tc.tile_pool(name="sing", bufs=1))
    wcp = ctx.enter_context(tc.tile_pool(name="wcp", bufs=2))
    qp = ctx.enter_context(tc.tile_pool(name="qp", bufs=1))
    sm = ctx.enter_context(tc.tile_pool(name="sm", bufs=2))
    ps_s = ctx.enter_context(tc.tile_pool(name="ps_s", bufs=2, space="PSUM"))
    ps_p = ctx.enter_context(tc.tile_pool(name="ps_p", bufs=2, space="PSUM"))
    ps_b = ctx.enter_context(tc.tile_pool(name="ps_b", bufs=2, space="PSUM"))

    # preload the exp table set during the DMA ramp
    actwarm = sing.tile([1, 1], fp32)
    nc.vector.memset(actwarm, 1.0)
    nc.scalar.activation(out=actwarm, in_=actwarm, func=AF.Exp)

    # ---- resident inputs ---------------------------------------------------
    xT_sb = sing.tile([128, T * B], bf16)
    xn_sb = sing.tile([B, T * 128], bf16)
    wp_sb = sing.tile([128, T * UV], bf16)
    s2_sb = sing.tile([128, 128], bf16)
    b_exp = sing.tile([128, BU], fp32)

    nc.sync.dma_start(out=s2_sb, in_=s2_d)
    # few, large chunks: each sync.dma_start costs ~0.6us of issue time
    nc.sync.dma_start(out=xT_sb[:, : 8 * B], in_=xT_d[:, : 8 * B])
    for c4 in range(4):
        w = 4 * UV
        nc.sync.dma_start(
            out=wp_sb[:, c4 * w : (c4 + 1) * w],
            in_=wp_d[:, c4 * w : (c4 + 1) * w],
        )
    nc.sync.dma_start(out=xT_sb[:, 8 * B :], in_=xT_d[:, 8 * B :])
    nc.sync.dma_start(out=xn_sb, in_=xn_d)
    nc.vector.memset(b_exp, 0.0)

    def reduce_s(s_psum, it):
        """Remote-DMA all-reduce of the [B,UV] n-partial sum.

        PSUM -> bf16 SBUF -> 7 relative-dest broadcasts (slot d, XOR
        bijection) -> fp32 accumulate of the 8 partials on DVE.  Returns
        the fp32 SBUF tile holding the full sum."""
        s_part = sing.tile([B, UV], bf16, name=f"s_part{it}")
        land = sing.tile([B, 7 * UV], bf16, name=f"land{it}")
        H = UV // 2
        nc.scalar.copy(out=s_part[:, :H], in_=s_psum[:, :H])
        nc.scalar.copy(out=s_part[:, H:], in_=s_psum[:, H:])
        for d in range(1, 8):
            rd = [None] * 8
            rd[d] = (0, d)
            nc.gpsimd.remote_dma_broadcast(
                land[:, (d - 1) * UV : d * UV], s_part[:, :],
                rsem, lsem, rdests=rd,
            )
        # No lsem wait: the 21-broadcast total fits the SWDGE ring;
        # receivers' rsem waits gate all data hazards.
        trig = nc.gpsimd.trigger_dma(count=None)
        if it == 0:
            # Entry barrier: the compile-time prelude AllGather both makes
            # NRT rendezvous the 8 core launches (a collective-free NEFF
            # starts cores up to ~10ms apart under the profiler) and
            # guarantees every peer has passed its per-execution semaphore
            # reset before the first remote send.  Registered here; the
            # wait itself is attached post-Tile (the scheduler sim cannot
            # model the prelude's increment).
            nc._bir_kernel_barrier_sem_replica_groups.append(set(range(NC)))
            post_waits.append((trig, nc._bir_kernel_barrier_sem, 1))
        s_sb = sm.tile([B, UV], fp32, tag="s_acc", name=f"s_acc{it}")
        a = nc.vector.tensor_add(s_sb, s_part, land[:, 0:UV])
        post_waits.append((a, rsem, AR_INC * (it + 1)))
        for d in range(2, 8):
            nc.vector.tensor_add(s_sb, s_sb, land[:, (d - 1) * UV : d * UV])
        return s_sb, s_part

    # ---- iteration 0: c uniform, s0 = sum_t xT_t.T @ wp_t ------------------
    s_psum = ps_s.tile([B, UV], fp32, tag="s")
    for t in range(T):
        nc.tensor.matmul(
            out=s_psum,
            lhsT=xT_sb[:, t * B : (t + 1) * B],
            rhs=wp_sb[:, t * UV : (t + 1) * UV],
            start=(t == 0),
            stop=(t == T - 1),
        )
    s_sb, s_part = reduce_s(s_psum, 0)

    # ---- post-AR fused pipeline for iterations 1 and 2 ---------------------
    warm_tiles = []
    for i in range(2):
        # PE clock warm-up: the HAM gate needs ~3.4us of dense activity to
        # lift the PE from 1.2 to 2.4 GHz.  These accumulating matmuls are
        # gated on the local bf16 partial, so they fill the otherwise
        # PE-idle all-reduce + squash window and the burst then runs warm.
        warm_ps = ps_b.tile([128, BU], fp32, tag="bd", name=f"warm{i}")
        for w in range(8):
            nc.tensor.matmul(out=warm_ps, lhsT=wp_sb[:, :B], rhs=s_part,
                             start=(w == 0), stop=(w == 7))
        warm_tiles.append(warm_ps)
        s_scale = 1.0 / U if i == 0 else 1.0
        v_bf = sm.tile([B, UV], bf16, tag="v_bf", name=f"v_bf{i}")
        _squash(nc, mybir, sm, fp32, bass, s_sb, v_bf, s_scale, B, f"i{i}",
                newton=1)

        q_sb = qp.tile([128, T * UV], bf16, tag="q")
        qr_bf = sm.tile([128, BU], bf16, tag="qr_bf")
        e_sb = sm.tile([128, BU], fp32, tag="e")
        rden = sm.tile([128, T], fp32, tag="rden")
        cx_sb = wcp.tile([128, T * UV], bf16, tag="cx")
        wc_sb = wcp.tile([128, T * UV], bf16, tag="wc")
        bd_psum = ps_b.tile([128, BU], fp32, tag="bd")
        s_psum = ps_s.tile([B, UV], fp32, tag="s")

        for g in range(NG):
            lo_t = g * GRP
            cbu = slice(g * GU, (g + 1) * GU)      # b/c columns
            cwc = slice(g * GUV, (g + 1) * GUV)    # wc columns
            # b_delta for these GRP tiles
            p_psum = ps_p.tile([128, GUV], fp32, tag="p")
            p_bf = sm.tile([128, GUV], bf16, tag="p_bf")
            for j in range(GRP):
                t = lo_t + j
                nc.tensor.matmul(
                    out=p_psum[:, j * UV : (j + 1) * UV],
                    lhsT=xn_sb[:, t * 128 : (t + 1) * 128],
                    rhs=v_bf,
                    start=True,
                    stop=True,
                )
            nc.scalar.copy(out=p_bf, in_=p_psum)
            nc.vector.tensor_mul(q_sb[:, cwc], p_bf, wp_sb[:, cwc])
            with nc.allow_low_precision(reason="qr feeds fp32 PSUM matmul"):
                nc.vector.reduce_sum(
                    out=qr_bf[:, cbu],
                    in_=q_sb[:, cwc].rearrange("p (a v) -> p a v", v=V),
                    axis=mybir.AxisListType.X,
                )
            nc.tensor.matmul(
                out=bd_psum[:, cbu], lhsT=s2_sb, rhs=qr_bf[:, cbu],
                start=True, stop=True,
            )
            nc.vector.tensor_add(b_exp[:, cbu], b_exp[:, cbu], bd_psum[:, cbu])
            # softmax for these tiles, folded into the weights
            nc.scalar.activation(out=e_sb[:, cbu], in_=b_exp[:, cbu],
                                 func=AF.Exp)
            nc.vector.reduce_sum(
                out=rden[:, g * GRP : (g + 1) * GRP],
                in_=e_sb[:, cbu].rearrange("p (t u) -> p t u", u=U),
                axis=mybir.AxisListType.X,
            )
            nc.vector.reciprocal(
                out=rden[:, g * GRP : (g + 1) * GRP],
                in_=rden[:, g * GRP : (g + 1) * GRP],
            )
            # c expanded over v on ScalarE (c = e * 1/den, per-partition
            # scale per tile); unit-stride bf16 result lets the Wc multiply
            # hit the DVE 2x mode
            for j in range(GRP):
                t = lo_t + j
                ev = e_sb[:, t * U : (t + 1) * U]
                e_b = bass.AP(tensor=ev.tensor, offset=ev.offset,
                              ap=[ev.ap[0], [1, U], [0, V]])
                cx_out = cx_sb[:, t * UV : (t + 1) * UV].rearrange(
                    "p (u v) -> p u v", v=V
                )
                nc.scalar.activation(out=cx_out, in_=e_b, func=AF.Copy,
                                     scale=rden[:, t : t + 1])
            nc.vector.tensor_mul(wc_sb[:, cwc], wp_sb[:, cwc], cx_sb[:, cwc])
            for j in range(GRP):
                t = lo_t + j
                nc.tensor.matmul(
                    out=s_psum,
                    lhsT=xT_sb[:, t * B : (t + 1) * B],
                    rhs=wc_sb[:, t * UV : (t + 1) * UV],
                    start=(t == 0),
                    stop=(t == T - 1),
                )
        s_sb, s_part = reduce_s(s_psum, i + 1)

    # ---- final squash: every core has the full sum; host reads core 0 ------
    v_sb = sm.tile([B, UV], fp32, tag="v_sb2")
    _squash(nc, mybir, sm, fp32, bass, s_sb, v_sb, 1.0, B, "2", newton=2)
    nc.sync.dma_start(out=vo_d, in_=v_sb)
    wseed = sm.tile([1, 8], fp32, tag="wseed")
    for k, wt in enumerate(warm_tiles):
        nc.scalar.copy(out=wseed[:, k * 4 : (k + 1) * 4], in_=wt[0:1, 0:4])
    nc.sync.dma_start(out=dbg_d, in_=wseed)

    ctx.close()


def _host_prep(x, W):
    """Slice + relayout the full inputs into the 8 per-core input maps."""
    bf = ml_dtypes.bfloat16
    S2 = np.zeros((128, 128), np.float32)
    for p in range(0, 128, 8):
        S2[p : p + 8, p : p + 8] = 1.0
    S2 = S2.astype(bf)
    maps = []
    for c in range(NC):
        sl = slice(c * NSH, (c + 1) * NSH)
        Wc = np.ascontiguousarray(W[sl])                    # [256,32,8,16]
        Wp = Wc.transpose(0, 2, 1, 3).reshape(NSH * K, UV)  # [(n k),(u v)]
        wp = Wp.reshape(T, 128, UV).transpose(1, 0, 2).reshape(128, T * UV)
        xc = np.ascontiguousarray(x[:, sl, :])              # [128,256,8]
        xn = xc.reshape(B, NSH * K)
        xT = (
            xc.transpose(1, 2, 0)
            .reshape(T, 128, B)
            .transpose(1, 0, 2)
            .reshape(128, T * B)
        )
        maps.append(
            {
                "xT": np.ascontiguousarray(xT).astype(bf),
                "xn": np.ascontiguousarray(xn).astype(bf),
                "wp": np.ascontiguousarray(wp).astype(bf),
                "s2": S2,
            }
        )
    return maps


def kernel(x: np.ndarray, W: np.ndarray) -> np.ndarray:
    from concourse.bass_utils import run_bass_kernel_spmd

    if "nc" not in _cache:
        _cache["nc"] = _build_program()
    nc = _cache["nc"]
    in_maps = _host_prep(np.asarray(x, np.float32), np.asarray(W, np.float32))
    res = run_bass_kernel_spmd(nc, in_maps, core_ids=list(range(NC)))
    return res.results[0]["vout"].reshape(B, U, V).astype(np.float32)


# revision 9
# speedup vs baseline: 59.1601x; 1.8886x over previous
"""CapsNet dense routing kernel for 8 Trainium2 NeuronCores.

Problem: capsule routing with 3 iterations (last skips the logit update).
  u_hat[b,n,u,v] = sum_k W[n,u,k,v] * x[b,n,k]        (B=128, N=2048, U=32, K=8, V=16)
  repeat:  c = softmax(b_logit, axis=u)
           s[b,u,v] = sum_n c[n,u] u_hat[b,n,u,v]
           v = squash(s)
           b_logit[n,u] += sum_{b,v} u_hat[b,n,u,v] v[b,u,v]

Strategy: shard n (in_caps) across the 8 cores (256 each).  u_hat is never
materialized:
  - s is one fused matmul  s[b,(uv)] = sum_(nk) xT[(nk),b] * (c*W)[(nk),(uv)]
    with only the n-partial sum needing a [128,512] AllReduce per iteration
    (iterations 0/1; the last uses ReduceScatter and the host concatenates
    the per-core row shards of the output).
  - the logit update uses P[(nk),(uv)] = sum_b x[b,(nk)] v[b,(uv)] (a matmul),
    then b_delta[n,u] = sum_{k,v} P*W  via an elementwise multiply, a
    v-reduction, and one block-diagonal "sum over k" matmul.
Matmul operands travel in bf16 (fp32 matmul costs two PE passes); all
accumulation is fp32.  sqrt inside squash uses a bitcast fast-rsqrt with two
Newton steps on the vector engine so ScalarE never leaves the exp table set.
"""

import sys

sys.path.insert(0, "/opt/trn_rl_repo")

import ml_dtypes
import numpy as np

B, N, U, K, V, NC = 128, 2048, 32, 8, 16, 8
NSH = N // NC            # 256 in_caps per core
T = NSH * K // 128       # 16 contraction tiles of 128 (n,k) rows
UV = U * V               # 512
BU = T * U               # 512 free size of the k-replicated logit buffer
GRP = 2                  # P-tiles per PSUM round
RS_P = B // NC           # 16 output rows per core from the ReduceScatter
WC_SPLIT = 13            # Wc tiles on DVE; rest on gpsimd (DVE ~3.4x faster)
RSQRT_MAGIC = 0x5F3759DF

_cache = {}


def _build_program():
    import concourse.mybir as mybir
    import concourse.tile as tile
    from concourse import bacc

    fp32 = mybir.dt.float32
    bf16 = mybir.dt.bfloat16

    nc = bacc.Bacc(
        "TRN2", target_bir_lowering=False, debug=False, num_devices=NC
    )
    xT_d = nc.dram_tensor("xT", [128, T * B], bf16, kind="ExternalInput").ap()
    xn_d = nc.dram_tensor("xn", [B, T * 128], bf16, kind="ExternalInput").ap()
    wp_d = nc.dram_tensor("wp", [128, T * UV], bf16, kind="ExternalInput").ap()
    s2_d = nc.dram_tensor("s2", [128, 128], bf16, kind="ExternalInput").ap()
    vo_d = nc.dram_tensor("vout", [RS_P, UV], fp32, kind="ExternalOutput").ap()
    dbg_d = nc.dram_tensor("dbg", [1, 8], fp32, kind="ExternalOutput").ap()

    with tile.TileContext(nc) as tc:
        _body(tc, nc, mybir, fp32, bf16, xT_d, xn_d, wp_d, s2_d, vo_d, dbg_d)
    nc.compile()
    return nc


def _squash(nc, mybir, sm, fp32, bass, s_in, v_out, s_scale, pp, tag,
            newton=2):
    """v_out = squash(s_in * s_scale); s_in [pp, UV] viewed [pp, U, V].

    factor = sqrt(n2)/(1+n2) with n2 = scale^2 * sum_v s^2; all on DVE:
    rsqrt seed by integer bitcast, Newton steps, no ScalarE tables.
    """
    Alu = mybir.AluOpType
    sq = sm.tile([pp, UV], fp32, tag=f"sq{tag}")
    n2 = sm.tile([pp, U], fp32, tag=f"n2{tag}")
    y = sm.tile([pp, U], fp32, tag=f"y{tag}")
    t = sm.tile([pp, U], fp32, tag=f"t{tag}")
    d_ = sm.tile([pp, U], fp32, tag=f"d_{tag}")
    rd_ = sm.tile([pp, U], fp32, tag=f"rd_{tag}")
    fac = sm.tile([pp, U], fp32, tag=f"fac{tag}")
    nc.vector.tensor_mul(sq[:, : UV // 2], s_in[:, : UV // 2],
                         s_in[:, : UV // 2])
    nc.vector.tensor_mul(sq[:, UV // 2 :], s_in[:, UV // 2 :],
                         s_in[:, UV // 2 :])
    nc.vector.reduce_sum(
        out=n2, in_=sq.rearrange("p (u v) -> p u v", v=V),
        axis=mybir.AxisListType.X,
    )
    if s_scale != 1.0:
        nc.vector.tensor_scalar_mul(out=n2, in0=n2,
                                    scalar1=float(s_scale * s_scale))
    # y0 = bitcast(0x5F3759DF - (bitcast(n2) >> 1))
    nc.vector.tensor_scalar(
        out=y.bitcast(mybir.dt.int32), in0=n2.bitcast(mybir.dt.int32),
        scalar1=1, scalar2=None, op0=Alu.logical_shift_right,
    )
    nc.vector.tensor_scalar(
        out=y.bitcast(mybir.dt.int32), in0=y.bitcast(mybir.dt.int32),
        scalar1=-1, scalar2=RSQRT_MAGIC, op0=Alu.mult, op1=Alu.add,
    )
    for _ in range(newton):  # Newton: y *= 1.5 - 0.5*n2*y*y
        nc.vector.tensor_mul(t, y, y)
        nc.vector.tensor_mul(t, t, n2)
        nc.vector.tensor_scalar(
            out=t, in0=t, scalar1=-0.5, scalar2=1.5, op0=Alu.mult, op1=Alu.add
        )
        nc.vector.tensor_mul(y, y, t)
    # d = 1+n2 ; fac = (n2*scale)*y / d      ((n2*y) = sqrt(n2))
    nc.vector.tensor_scalar_add(out=d_, in0=n2, scalar1=1.0)
    nc.vector.reciprocal(out=rd_, in_=d_)
    nc.vector.scalar_tensor_tensor(
        out=t, in0=n2, scalar=float(s_scale), in1=y,
        op0=Alu.mult, op1=Alu.mult,
    )
    nc.vector.tensor_mul(fac, t, rd_)
    fac_b = bass.AP(
        tensor=fac.tensor, offset=fac.offset,
        ap=[fac.ap[0], [1, U], [0, V]],
    )
    nc.vector.tensor_mul(
        v_out.rearrange("p (u v) -> p u v", v=V),
        s_in.rearrange("p (u v) -> p u v", v=V),
        fac_b,
    )


def _body(tc, nc, mybir, fp32, bf16, xT_d, xn_d, wp_d, s2_d, vo_d, dbg_d):
    from contextlib import ExitStack

    import concourse.bass as bass

    AF = mybir.ActivationFunctionType
    rg = [list(range(NC))]
    NG = T // GRP            # fused pipeline groups per iteration
    GU = GRP * U             # 64  b-logit columns per group
    GUV = GRP * UV           # 1024 wc columns per group

    ctx = ExitStack()
    tc._caps_ctx = ctx
    sing = ctx.enter_context(tc.tile_pool(name="sing", bufs=1))
    wcp = ctx.enter_context(tc.tile_pool(name="wcp", bufs=2))
    qp = ctx.enter_context(tc.tile_pool(name="qp", bufs=1))
    sm = ctx.enter_context(tc.tile_pool(name="sm", bufs=2))
    ps_s = ctx.enter_context(tc.tile_pool(name="ps_s", bufs=2, space="PSUM"))
    ps_p = ctx.enter_context(tc.tile_pool(name="ps_p", bufs=2, space="PSUM"))
    ps_b = ctx.enter_context(tc.tile_pool(name="ps_b", bufs=2, space="PSUM"))
    dram = ctx.enter_context(tc.tile_pool(name="dram", bufs=1, space="DRAM"))

    # preload the exp table set during the DMA ramp
    actwarm = sing.tile([1, 1], fp32)
    nc.vector.memset(actwarm, 1.0)
    nc.scalar.activation(out=actwarm, in_=actwarm, func=AF.Exp)

    # ncfw warm-up: the first collective of an execution pays ~35us of
    # TOPSP firmware wakeup before any data moves.  Issue a throwaway
    # 4-byte AllReduce immediately so that cost overlaps the input DMA +
    # iter-0 matmul instead of sitting on the critical path of the first
    # real all-reduce.  Nothing waits on it; the CC stream is FIFO per
    # core so the real AR is processed right after it with ncfw warm.
    ccw_in = dram.tile([1, 1], fp32, name="ccw_in")
    ccw_out = dram.tile([1, 1], fp32, name="ccw_out", addr_space="Shared")
    ccw_sb = sing.tile([1, 1], fp32)
    nc.vector.memset(ccw_sb, 0.0)
    nc.sync.dma_start(out=ccw_in, in_=ccw_sb)
    nc.gpsimd.collective_compute(
        "AllReduce", mybir.AluOpType.add, replica_groups=rg,
        ins=[ccw_in.opt()], outs=[ccw_out.opt()],
    )

    # ---- resident inputs ---------------------------------------------------
    xT_sb = sing.tile([128, T * B], bf16)
    xn_sb = sing.tile([B, T * 128], bf16)
    wp_sb = sing.tile([128, T * UV], bf16)
    s2_sb = sing.tile([128, 128], bf16)
    b_exp = sing.tile([128, BU], fp32)

    nc.sync.dma_start(out=s2_sb, in_=s2_d)
    # few, large chunks: each sync.dma_start costs ~0.6us of issue time
    nc.sync.dma_start(out=xT_sb[:, : 8 * B], in_=xT_d[:, : 8 * B])
    for c4 in range(4):
        w = 4 * UV
        nc.sync.dma_start(
            out=wp_sb[:, c4 * w : (c4 + 1) * w],
            in_=wp_d[:, c4 * w : (c4 + 1) * w],
        )
    nc.sync.dma_start(out=xT_sb[:, 8 * B :], in_=xT_d[:, 8 * B :])
    nc.sync.dma_start(out=xn_sb, in_=xn_d)
    nc.vector.memset(b_exp, 0.0)

    def reduce_s(s_psum, it):
        """PSUM -> SBUF -> DRAM -> collective.  Returns the SBUF landing tile
        of the reduced result (bf16 AR for it<2, fp32 ReduceScatter for it=2)."""
        last = it == 2
        ar_dt = fp32 if last else bf16
        sp_d = dram.tile([B, UV], ar_dt, tag=f"spart{int(last)}",
                         name=f"spart{it}")
        s_part_sb = sm.tile([B, UV], ar_dt, tag=f"s_part{int(last)}")
        H = UV // 2
        nc.scalar.copy(out=s_part_sb[:, :H], in_=s_psum[:, :H])
        nc.sync.dma_start(out=sp_d[:, :H], in_=s_part_sb[:, :H])
        nc.scalar.copy(out=s_part_sb[:, H:], in_=s_psum[:, H:])
        nc.sync.dma_start(out=sp_d[:, H:], in_=s_part_sb[:, H:])
        if last:
            sg_d = dram.tile([RS_P, UV], fp32, name="sglob2")
            nc.gpsimd.collective_compute(
                "ReduceScatter", mybir.AluOpType.add, replica_groups=rg,
                ins=[sp_d.opt()], outs=[sg_d.opt()],
            )
            s_sb = sm.tile([RS_P, UV], fp32, tag="s_sb2")
        else:
            sg_d = dram.tile([B, UV], bf16, tag="sglob", name=f"sglob{it}",
                             addr_space="Shared")
            nc.gpsimd.collective_compute(
                "AllReduce", mybir.AluOpType.add, replica_groups=rg,
                ins=[sp_d.opt()], outs=[sg_d.opt()],
            )
            s_sb = sm.tile([B, UV], bf16, tag="s_sb", name=f"s_sb{it}")
            nc.sync.dma_start(out=s_sb[:, : UV // 2], in_=sg_d[:, : UV // 2])
            nc.sync.dma_start(out=s_sb[:, UV // 2 :], in_=sg_d[:, UV // 2 :])
            return s_sb
        nc.sync.dma_start(out=s_sb, in_=sg_d)
        return s_sb

    # ---- iteration 0: c uniform, s0 = sum_t xT_t.T @ wp_t ------------------
    s_psum = ps_s.tile([B, UV], fp32, tag="s")
    for t in range(T):
        nc.tensor.matmul(
            out=s_psum,
            lhsT=xT_sb[:, t * B : (t + 1) * B],
            rhs=wp_sb[:, t * UV : (t + 1) * UV],
            start=(t == 0),
            stop=(t == T - 1),
        )
    s_sb = reduce_s(s_psum, 0)

    # ---- post-AR fused pipeline for iterations 1 and 2 ---------------------
    warm_tiles = []
    for i in range(2):
        # PE clock warm-up: the HAM gate needs ~3.4us of dense activity to
        # lift the PE from 1.2 to 2.4 GHz.  These accumulating matmuls are
        # gated on the AllReduce landing DMA, so they fill the otherwise
        # PE-idle squash window and the gap then runs warm.
        warm_ps = ps_b.tile([128, BU], fp32, tag="bd", name=f"warm{i}")
        for w in range(8):
            nc.tensor.matmul(out=warm_ps, lhsT=wp_sb[:, :B], rhs=s_sb,
                             start=(w == 0), stop=(w == 7))
        warm_tiles.append(warm_ps)
        s_scale = 1.0 / U if i == 0 else 1.0
        v_bf = sm.tile([B, UV], bf16, tag="v_bf", name=f"v_bf{i}")
        _squash(nc, mybir, sm, fp32, bass, s_sb, v_bf, s_scale, B, f"i{i}",
                newton=1)

        q_sb = qp.tile([128, T * UV], bf16, tag="q")
        qr_bf = sm.tile([128, BU], bf16, tag="qr_bf")
        e_sb = sm.tile([128, BU], fp32, tag="e")
        rden = sm.tile([128, T], fp32, tag="rden")
        cx_sb = wcp.tile([128, T * UV], bf16, tag="cx")
        wc_sb = wcp.tile([128, T * UV], bf16, tag="wc")
        bd_psum = ps_b.tile([128, BU], fp32, tag="bd")
        s_psum = ps_s.tile([B, UV], fp32, tag="s")

        for g in range(NG):
            lo_t = g * GRP
            cbu = slice(g * GU, (g + 1) * GU)      # b/c columns
            cwc = slice(g * GUV, (g + 1) * GUV)    # wc columns
            # b_delta for these GRP tiles
            p_psum = ps_p.tile([128, GUV], fp32, tag="p")
            p_bf = sm.tile([128, GUV], bf16, tag="p_bf")
            for j in range(GRP):
                t = lo_t + j
                nc.tensor.matmul(
                    out=p_psum[:, j * UV : (j + 1) * UV],
                    lhsT=xn_sb[:, t * 128 : (t + 1) * 128],
                    rhs=v_bf,
                    start=True,
                    stop=True,
                )
            nc.scalar.copy(out=p_bf, in_=p_psum)
            nc.vector.tensor_mul(q_sb[:, cwc], p_bf, wp_sb[:, cwc])
            with nc.allow_low_precision(reason="qr feeds fp32 PSUM matmul"):
                nc.vector.reduce_sum(
                    out=qr_bf[:, cbu],
                    in_=q_sb[:, cwc].rearrange("p (a v) -> p a v", v=V),
                    axis=mybir.AxisListType.X,
                )
            nc.tensor.matmul(
                out=bd_psum[:, cbu], lhsT=s2_sb, rhs=qr_bf[:, cbu],
                start=True, stop=True,
            )
            nc.vector.tensor_add(b_exp[:, cbu], b_exp[:, cbu], bd_psum[:, cbu])
            # softmax for these tiles, folded into the weights
            nc.scalar.activation(out=e_sb[:, cbu], in_=b_exp[:, cbu],
                                 func=AF.Exp)
            nc.vector.reduce_sum(
                out=rden[:, g * GRP : (g + 1) * GRP],
                in_=e_sb[:, cbu].rearrange("p (t u) -> p t u", u=U),
                axis=mybir.AxisListType.X,
            )
            nc.vector.reciprocal(
                out=rden[:, g * GRP : (g + 1) * GRP],
                in_=rden[:, g * GRP : (g + 1) * GRP],
            )
            # c expanded over v on ScalarE (c = e * 1/den, per-partition
            # scale per tile); unit-stride bf16 result lets the Wc multiply
            # hit the DVE 2x mode
            for j in range(GRP):
                t = lo_t + j
                ev = e_sb[:, t * U : (t + 1) * U]
                e_b = bass.AP(tensor=ev.tensor, offset=ev.offset,
                              ap=[ev.ap[0], [1, U], [0, V]])
                cx_out = cx_sb[:, t * UV : (t + 1) * UV].rearrange(
                    "p (u v) -> p u v", v=V
                )
                nc.scalar.activation(out=cx_out, in_=e_b, func=AF.Copy,
                                     scale=rden[:, t : t + 1])
            nc.vector.tensor_mul(wc_sb[:, cwc], wp_sb[:, cwc], cx_sb[:, cwc])
            for j in range(GRP):
                t = lo_t + j
                nc.tensor.matmul(
                    out=s_psum,
                    lhsT=xT_sb[:, t * B : (t + 1) * B],
                    rhs=wc_sb[:, t * UV : (t + 1) * UV],
                    start=(t == 0),
                    stop=(t == T - 1),
                )
        s_sb = reduce_s(s_psum, i + 1)

    # ---- final squash on this core's ReduceScatter shard -------------------
    v_sb = sm.tile([RS_P, UV], fp32, tag="v_sb2")
    _squash(nc, mybir, sm, fp32, bass, s_sb, v_sb, 1.0, RS_P, "2", newton=2)
    nc.sync.dma_start(out=vo_d, in_=v_sb)
    wseed = sm.tile([1, 8], fp32, tag="wseed")
    for k, wt in enumerate(warm_tiles):
        nc.scalar.copy(out=wseed[:, k * 4 : (k + 1) * 4], in_=wt[0:1, 0:4])
    nc.sync.dma_start(out=dbg_d, in_=wseed)

    ctx.close()


def _host_prep(x, W):
    """Slice + relayout the full inputs into the 8 per-core input maps."""
    bf = ml_dtypes.bfloat16
    S2 = np.zeros((128, 128), np.float32)
    for p in range(0, 128, 8):
        S2[p : p + 8, p : p + 8] = 1.0
    S2 = S2.astype(bf)
    maps = []
    for c in range(NC):
        sl = slice(c * NSH, (c + 1) * NSH)
        Wc = np.ascontiguousarray(W[sl])                    # [256,32,8,16]
        Wp = Wc.transpose(0, 2, 1, 3).reshape(NSH * K, UV)  # [(n k),(u v)]
        wp = Wp.reshape(T, 128, UV).transpose(1, 0, 2).reshape(128, T * UV)
        xc = np.ascontiguousarray(x[:, sl, :])              # [128,256,8]
        xn = xc.reshape(B, NSH * K)
        xT = (
            xc.transpose(1, 2, 0)
            .reshape(T, 128, B)
            .transpose(1, 0, 2)
            .reshape(128, T * B)
        )
        maps.append(
            {
                "xT": np.ascontiguousarray(xT).astype(bf),
                "xn": np.ascontiguousarray(xn).astype(bf),
                "wp": np.ascontiguousarray(wp).astype(bf),
                "s2": S2,
            }
        )
    return maps


def kernel(x: np.ndarray, W: np.ndarray) -> np.ndarray:
    from concourse.bass_utils import run_bass_kernel_spmd

    if "nc" not in _cache:
        _cache["nc"] = _build_program()
    nc = _cache["nc"]
    in_maps = _host_prep(np.asarray(x, np.float32), np.asarray(W, np.float32))
    res = run_bass_kernel_spmd(nc, in_maps, core_ids=list(range(NC)))
    out = np.concatenate(
        [res.results[c]["vout"] for c in range(NC)], axis=0
    )
    return out.reshape(B, U, V).astype(np.float32)



# revision 17
# speedup vs baseline: 61.6814x; 1.0426x over previous
"""CapsNet dense routing kernel for 8 Trainium2 NeuronCores.

Problem: capsule routing with 3 iterations (last skips the logit update).
  u_hat[b,n,u,v] = sum_k W[n,u,k,v] * x[b,n,k]        (B=128, N=2048, U=32, K=8, V=16)
  repeat:  c = softmax(b_logit, axis=u)
           s[b,u,v] = sum_n c[n,u] u_hat[b,n,u,v]
           v = squash(s)
           b_logit[n,u] += sum_{b,v} u_hat[b,n,u,v] v[b,u,v]

Strategy: shard n (in_caps) across the 8 cores (256 each).  u_hat is never
materialized:
  - s is one fused matmul  s[b,(uv)] = sum_(nk) xT[(nk),b] * (c*W)[(nk),(uv)]
    with only the n-partial sum needing a [128,512] AllReduce per iteration
    (iterations 0/1; the last uses ReduceScatter and the host concatenates
    the per-core row shards of the output).
  - the logit update uses P[(nk),(uv)] = sum_b x[b,(nk)] v[b,(uv)] (a matmul),
    then b_delta[n,u] = sum_{k,v} P*W  via an elementwise multiply, a
    v-reduction, and one block-diagonal "sum over k" matmul.
Matmul operands travel in bf16 (fp32 matmul costs two PE passes); all
accumulation is fp32.  sqrt inside squash uses a bitcast fast-rsqrt with two
Newton steps on the vector engine so ScalarE never leaves the exp table set.
"""

import sys

sys.path.insert(0, "/opt/trn_rl_repo")

import ml_dtypes
import numpy as np

B, N, U, K, V, NC = 128, 2048, 32, 8, 16, 8
NSH = N // NC            # 256 in_caps per core
T = NSH * K // 128       # 16 contraction tiles of 128 (n,k) rows
UV = U * V               # 512
BU = T * U               # 512 free size of the k-replicated logit buffer
GRP = 2                  # P-tiles per PSUM round
RS_P = B // NC           # 16 output rows per core from the ReduceScatter
WC_SPLIT = 13            # Wc tiles on DVE; rest on gpsimd (DVE ~3.4x faster)
RSQRT_MAGIC = 0x5F3759DF

_cache = {}


def _build_program():
    import concourse.mybir as mybir
    import concourse.tile as tile
    from concourse import bacc

    fp32 = mybir.dt.float32
    bf16 = mybir.dt.bfloat16

    nc = bacc.Bacc(
        "TRN2", target_bir_lowering=False, debug=False, num_devices=NC
    )
    xT_d = nc.dram_tensor("xT", [128, T * B], bf16, kind="ExternalInput").ap()
    xn_d = nc.dram_tensor("xn", [B, T * 128], bf16, kind="ExternalInput").ap()
    wp_d = nc.dram_tensor("wp", [128, T * UV], bf16, kind="ExternalInput").ap()
    s2_d = nc.dram_tensor("s2", [128, 128], bf16, kind="ExternalInput").ap()
    vo_d = nc.dram_tensor("vout", [RS_P, UV], fp32, kind="ExternalOutput").ap()
    dbg_d = nc.dram_tensor("dbg", [1, 8], fp32, kind="ExternalOutput").ap()

    with tile.TileContext(nc) as tc:
        _body(tc, nc, mybir, fp32, bf16, xT_d, xn_d, wp_d, s2_d, vo_d, dbg_d)
    nc.compile()
    return nc


def _squash(nc, mybir, sm, fp32, bass, s_in, v_out, s_scale, pp, tag,
            newton=2):
    """v_out = squash(s_in * s_scale); s_in [pp, UV] viewed [pp, U, V].

    factor = sqrt(n2)/(1+n2) with n2 = scale^2 * sum_v s^2; all on DVE:
    rsqrt seed by integer bitcast, Newton steps, no ScalarE tables.
    """
    Alu = mybir.AluOpType
    sq = sm.tile([pp, UV], fp32, tag=f"sq{tag}")
    n2 = sm.tile([pp, U], fp32, tag=f"n2{tag}")
    y = sm.tile([pp, U], fp32, tag=f"y{tag}")
    t = sm.tile([pp, U], fp32, tag=f"t{tag}")
    d_ = sm.tile([pp, U], fp32, tag=f"d_{tag}")
    rd_ = sm.tile([pp, U], fp32, tag=f"rd_{tag}")
    fac = sm.tile([pp, U], fp32, tag=f"fac{tag}")
    nc.vector.tensor_mul(sq[:, : UV // 2], s_in[:, : UV // 2],
                         s_in[:, : UV // 2])
    nc.vector.tensor_mul(sq[:, UV // 2 :], s_in[:, UV // 2 :],
                         s_in[:, UV // 2 :])
    nc.vector.reduce_sum(
        out=n2, in_=sq.rearrange("p (u v) -> p u v", v=V),
        axis=mybir.AxisListType.X,
    )
    if s_scale != 1.0:
        nc.vector.tensor_scalar_mul(out=n2, in0=n2,
                                    scalar1=float(s_scale * s_scale))
    # y0 = bitcast(0x5F3759DF - (bitcast(n2) >> 1))
    nc.vector.tensor_scalar(
        out=y.bitcast(mybir.dt.int32), in0=n2.bitcast(mybir.dt.int32),
        scalar1=1, scalar2=None, op0=Alu.logical_shift_right,
    )
    nc.vector.tensor_scalar(
        out=y.bitcast(mybir.dt.int32), in0=y.bitcast(mybir.dt.int32),
        scalar1=-1, scalar2=RSQRT_MAGIC, op0=Alu.mult, op1=Alu.add,
    )
    for _ in range(newton):  # Newton: y *= 1.5 - 0.5*n2*y*y
        nc.vector.tensor_mul(t, y, y)
        nc.vector.tensor_mul(t, t, n2)
        nc.vector.tensor_scalar(
            out=t, in0=t, scalar1=-0.5, scalar2=1.5, op0=Alu.mult, op1=Alu.add
        )
        nc.vector.tensor_mul(y, y, t)
    # d = 1+n2 ; fac = (n2*scale)*y / d      ((n2*y) = sqrt(n2))
    nc.vector.tensor_scalar_add(out=d_, in0=n2, scalar1=1.0)
    nc.vector.reciprocal(out=rd_, in_=d_)
    nc.vector.scalar_tensor_tensor(
        out=t, in0=n2, scalar=float(s_scale), in1=y,
        op0=Alu.mult, op1=Alu.mult,
    )
    nc.vector.tensor_mul(fac, t, rd_)
    fac_b = bass.AP(
        tensor=fac.tensor, offset=fac.offset,
        ap=[fac.ap[0], [1, U], [0, V]],
    )
    nc.vector.tensor_mul(
        v_out.rearrange("p (u v) -> p u v", v=V),
        s_in.rearrange("p (u v) -> p u v", v=V),
        fac_b,
    )


def _body(tc, nc, mybir, fp32, bf16, xT_d, xn_d, wp_d, s2_d, vo_d, dbg_d):
    from contextlib import ExitStack

    import concourse.bass as bass

    AF = mybir.ActivationFunctionType
    rg = [list(range(NC))]
    NG = T // GRP            # fused pipeline groups per iteration
    GU = GRP * U             # 64  b-logit columns per group
    GUV = GRP * UV           # 1024 wc columns per group

    ctx = ExitStack()
    tc._caps_ctx = ctx
    sing = ctx.enter_context(tc.tile_pool(name="sing", bufs=1))
    wcp = ctx.enter_context(tc.tile_pool(name="wcp", bufs=2))
    qp = ctx.enter_context(tc.tile_pool(name="qp", bufs=1))
    sm = ctx.enter_context(tc.tile_pool(name="sm", bufs=2))
    ps_s = ctx.enter_context(tc.tile_pool(name="ps_s", bufs=2, space="PSUM"))
    ps_p = ctx.enter_context(tc.tile_pool(name="ps_p", bufs=2, space="PSUM"))
    ps_b = ctx.enter_context(tc.tile_pool(name="ps_b", bufs=2, space="PSUM"))
    dram = ctx.enter_context(tc.tile_pool(name="dram", bufs=1, space="DRAM"))

    # preload the exp table set during the DMA ramp
    actwarm = sing.tile([1, 1], fp32)
    nc.vector.memset(actwarm, 1.0)
    nc.scalar.activation(out=actwarm, in_=actwarm, func=AF.Exp)

    # ncfw warm-up: the first collective of an execution pays ~40us of
    # control-plane latency before its data moves, and the cost is per
    # message-size-class.  Issue a garbage AllReduce of the SAME shape as
    # the real one immediately, so wakeup + size-class setup overlap the
    # input DMA + iter-0 matmul.  Nothing waits on it; the CC stream is
    # FIFO per core so the real AR runs right behind it, warm.
    ccw_in = dram.tile([B, UV], bf16, name="ccw_in")
    ccw_out = dram.tile([B, UV], bf16, name="ccw_out", addr_space="Shared")
    nc.gpsimd.collective_compute(
        "AllReduce", mybir.AluOpType.add, replica_groups=rg,
        ins=[ccw_in.opt()], outs=[ccw_out.opt()],
    )

    # ---- resident inputs ---------------------------------------------------
    xT_sb = sing.tile([128, T * B], bf16)
    xn_sb = sing.tile([B, T * 128], bf16)
    wp_sb = sing.tile([128, T * UV], bf16)
    s2_sb = sing.tile([128, 128], bf16)
    b_exp = sing.tile([128, BU], fp32)

    nc.sync.dma_start(out=s2_sb, in_=s2_d)
    # few, large chunks: each sync.dma_start costs ~0.6us of issue time
    nc.sync.dma_start(out=xT_sb[:, : 8 * B], in_=xT_d[:, : 8 * B])
    for c4 in range(4):
        w = 4 * UV
        nc.sync.dma_start(
            out=wp_sb[:, c4 * w : (c4 + 1) * w],
            in_=wp_d[:, c4 * w : (c4 + 1) * w],
        )
    nc.sync.dma_start(out=xT_sb[:, 8 * B :], in_=xT_d[:, 8 * B :])
    nc.sync.dma_start(out=xn_sb, in_=xn_d)
    nc.vector.memset(b_exp, 0.0)

    def reduce_s(s_psum, it):
        """PSUM -> SBUF -> DRAM -> collective.  Returns the SBUF landing tile
        of the reduced result (bf16 AR for it<2, fp32 ReduceScatter for it=2)."""
        last = it == 2
        ar_dt = bf16
        sp_d = dram.tile([B, UV], ar_dt, tag=f"spart{int(last)}",
                         name=f"spart{it}")
        s_part_sb = sm.tile([B, UV], ar_dt, tag=f"s_part{int(last)}")
        H = UV // 2
        nc.scalar.copy(out=s_part_sb[:, :H], in_=s_psum[:, :H])
        nc.sync.dma_start(out=sp_d[:, :H], in_=s_part_sb[:, :H])
        nc.scalar.copy(out=s_part_sb[:, H:], in_=s_psum[:, H:])
        nc.sync.dma_start(out=sp_d[:, H:], in_=s_part_sb[:, H:])
        if last:
            sg_d = dram.tile([RS_P, UV], ar_dt, name="sglob2")
            nc.gpsimd.collective_compute(
                "ReduceScatter", mybir.AluOpType.add, replica_groups=rg,
                ins=[sp_d.opt()], outs=[sg_d.opt()],
            )
            s_sb = sm.tile([RS_P, UV], ar_dt, tag="s_sb2")
        else:
            sg_d = dram.tile([B, UV], bf16, tag="sglob", name=f"sglob{it}",
                             addr_space="Shared")
            nc.gpsimd.collective_compute(
                "AllReduce", mybir.AluOpType.add, replica_groups=rg,
                ins=[sp_d.opt()], outs=[sg_d.opt()],
            )
            s_sb = sm.tile([B, UV], bf16, tag="s_sb", name=f"s_sb{it}")
            nc.sync.dma_start(out=s_sb[:, : UV // 2], in_=sg_d[:, : UV // 2])
            nc.sync.dma_start(out=s_sb[:, UV // 2 :], in_=sg_d[:, UV // 2 :])
            return s_sb
        nc.sync.dma_start(out=s_sb, in_=sg_d)
        return s_sb

    # ---- iteration 0: c uniform, s0 = sum_t xT_t.T @ wp_t ------------------
    s_psum = ps_s.tile([B, UV], fp32, tag="s")
    for t in range(T):
        nc.tensor.matmul(
            out=s_psum,
            lhsT=xT_sb[:, t * B : (t + 1) * B],
            rhs=wp_sb[:, t * UV : (t + 1) * UV],
            start=(t == 0),
            stop=(t == T - 1),
        )
    s_sb = reduce_s(s_psum, 0)

    # ---- post-AR fused pipeline for iterations 1 and 2 ---------------------
    warm_tiles = []
    for i in range(2):
        # PE clock warm-up: the HAM gate needs ~3.4us of dense activity to
        # lift the PE from 1.2 to 2.4 GHz.  These accumulating matmuls are
        # gated on the AllReduce landing DMA, so they fill the otherwise
        # PE-idle squash window and the gap then runs warm.
        warm_ps = ps_b.tile([128, BU], fp32, tag="bd", name=f"warm{i}")
        for w in range(8):
            nc.tensor.matmul(out=warm_ps, lhsT=wp_sb[:, :B], rhs=s_sb,
                             start=(w == 0), stop=(w == 7))
        warm_tiles.append(warm_ps)
        s_scale = 1.0 / U if i == 0 else 1.0
        v_bf = sm.tile([B, UV], bf16, tag="v_bf", name=f"v_bf{i}")
        _squash(nc, mybir, sm, fp32, bass, s_sb, v_bf, s_scale, B, f"i{i}",
                newton=1)

        q_sb = qp.tile([128, T * UV], bf16, tag="q")
        qr_bf = sm.tile([128, BU], bf16, tag="qr_bf")
        e_sb = sm.tile([128, BU], fp32, tag="e")
        rden = sm.tile([128, T], fp32, tag="rden")
        cx_sb = wcp.tile([128, T * UV], bf16, tag="cx")
        wc_sb = wcp.tile([128, T * UV], bf16, tag="wc")
        bd_psum = ps_b.tile([128, BU], fp32, tag="bd")
        s_psum = ps_s.tile([B, UV], fp32, tag="s")

        for g in range(NG):
            lo_t = g * GRP
            cbu = slice(g * GU, (g + 1) * GU)      # b/c columns
            cwc = slice(g * GUV, (g + 1) * GUV)    # wc columns
            # b_delta for these GRP tiles
            p_psum = ps_p.tile([128, GUV], fp32, tag="p")
            p_bf = sm.tile([128, GUV], bf16, tag="p_bf")
            for j in range(GRP):
                t = lo_t + j
                nc.tensor.matmul(
                    out=p_psum[:, j * UV : (j + 1) * UV],
                    lhsT=xn_sb[:, t * 128 : (t + 1) * 128],
                    rhs=v_bf,
                    start=True,
                    stop=True,
                )
            nc.scalar.copy(out=p_bf, in_=p_psum)
            QS = 768          # q columns on DVE; rest on gpsimd (SBUF-only)
            nc.vector.tensor_mul(q_sb[:, g * GUV : g * GUV + QS],
                                 p_bf[:, :QS], wp_sb[:, g * GUV : g * GUV + QS])
            nc.gpsimd.tensor_mul(q_sb[:, g * GUV + QS : (g + 1) * GUV],
                                 p_bf[:, QS:],
                                 wp_sb[:, g * GUV + QS : (g + 1) * GUV])
            with nc.allow_low_precision(reason="qr feeds fp32 PSUM matmul"):
                nc.vector.reduce_sum(
                    out=qr_bf[:, cbu],
                    in_=q_sb[:, cwc].rearrange("p (a v) -> p a v", v=V),
                    axis=mybir.AxisListType.X,
                )
            nc.tensor.matmul(
                out=bd_psum[:, cbu], lhsT=s2_sb, rhs=qr_bf[:, cbu],
                start=True, stop=True,
            )
            nc.vector.tensor_add(b_exp[:, cbu], b_exp[:, cbu],
                                 bd_psum[:, cbu])
            # softmax for these tiles, folded into the weights
            nc.scalar.activation(out=e_sb[:, cbu], in_=b_exp[:, cbu],
                                 func=AF.Exp)
            nc.vector.reduce_sum(
                out=rden[:, g * GRP : (g + 1) * GRP],
                in_=e_sb[:, cbu].rearrange("p (t u) -> p t u", u=U),
                axis=mybir.AxisListType.X,
            )
            nc.vector.reciprocal(
                out=rden[:, g * GRP : (g + 1) * GRP],
                in_=rden[:, g * GRP : (g + 1) * GRP],
            )
            # c expanded over v on ScalarE (c = e * 1/den, per-partition
            # scale per tile); unit-stride bf16 result lets the Wc multiply
            # hit the DVE 2x mode
            for j in range(GRP):
                t = lo_t + j
                ev = e_sb[:, t * U : (t + 1) * U]
                e_b = bass.AP(tensor=ev.tensor, offset=ev.offset,
                              ap=[ev.ap[0], [1, U], [0, V]])
                cx_out = cx_sb[:, t * UV : (t + 1) * UV].rearrange(
                    "p (u v) -> p u v", v=V
                )
                if j == 0:
                    nc.scalar.activation(out=cx_out, in_=e_b, func=AF.Copy,
                                         scale=rden[:, t : t + 1])
                else:
                    # second expansion of the group on gpsimd (SBUF-only):
                    # cx = e * (1/den), both broadcast along v
                    rv = rden[:, t : t + 1]
                    r_b = bass.AP(tensor=rv.tensor, offset=rv.offset,
                                  ap=[rv.ap[0], [0, U], [0, V]])
                    nc.gpsimd.tensor_mul(cx_out, e_b, r_b)
            nc.vector.tensor_mul(wc_sb[:, cwc], wp_sb[:, cwc], cx_sb[:, cwc])
            for j in range(GRP):
                t = lo_t + j
                nc.tensor.matmul(
                    out=s_psum,
                    lhsT=xT_sb[:, t * B : (t + 1) * B],
                    rhs=wc_sb[:, t * UV : (t + 1) * UV],
                    start=(t == 0),
                    stop=(t == T - 1),
                )
        s_sb = reduce_s(s_psum, i + 1)

    # ---- final squash on this core's ReduceScatter shard -------------------
    v_sb = sm.tile([RS_P, UV], fp32, tag="v_sb2")
    _squash(nc, mybir, sm, fp32, bass, s_sb, v_sb, 1.0, RS_P, "2", newton=2)
    nc.sync.dma_start(out=vo_d, in_=v_sb)
    wseed = sm.tile([1, 8], fp32, tag="wseed")
    for k, wt in enumerate(warm_tiles):
        nc.scalar.copy(out=wseed[:, k * 4 : (k + 1) * 4], in_=wt[0:1, 0:4])
    nc.sync.dma_start(out=dbg_d, in_=wseed)

    ctx.close()


def _host_prep(x, W):
    """Slice + relayout the full inputs into the 8 per-core input maps."""
    bf = ml_dtypes.bfloat16
    S2 = np.zeros((128, 128), np.float32)
    for p in range(0, 128, 8):
        S2[p : p + 8, p : p + 8] = 1.0
    S2 = S2.astype(bf)
    maps = []
    for c in range(NC):
        sl = slice(c * NSH, (c + 1) * NSH)
        Wc = np.ascontiguousarray(W[sl])                    # [256,32,8,16]
        Wp = Wc.transpose(0, 2, 1, 3).reshape(NSH * K, UV)  # [(n k),(u v)]
        wp = Wp.reshape(T, 128, UV).transpose(1, 0, 2).reshape(128, T * UV)
        xc = np.ascontiguousarray(x[:, sl, :])              # [128,256,8]
        xn = xc.reshape(B, NSH * K)
        xT = (
            xc.transpose(1, 2, 0)
            .reshape(T, 128, B)
            .transpose(1, 0, 2)
            .reshape(128, T * B)
        )
        maps.append(
            {
                "xT": np.ascontiguousarray(xT).astype(bf),
                "xn": np.ascontiguousarray(xn).astype(bf),
                "wp": np.ascontiguousarray(wp).astype(bf),
                "s2": S2,
            }
        )
    return maps


def kernel(x: np.ndarray, W: np.ndarray) -> np.ndarray:
    from concourse.bass_utils import run_bass_kernel_spmd

    if "nc" not in _cache:
        _cache["nc"] = _build_program()
    nc = _cache["nc"]
    in_maps = _host_prep(np.asarray(x, np.float32), np.asarray(W, np.float32))
    res = run_bass_kernel_spmd(nc, in_maps, core_ids=list(range(NC)))
    out = np.concatenate(
        [res.results[c]["vout"] for c in range(NC)], axis=0
    )
    return out.reshape(B, U, V).astype(np.float32)

